# revision 2
# baseline (speedup 1.0000x reference)
"""AttentionPool (segment softmax + weighted scatter-add) on 8 trn2 NeuronCores.

Strategy
--------
Segment-ALIGNED sharding: batch ids are sorted, and B = 1024 = 8 * 128, so
core c owns segments [128c, 128(c+1)) exactly.  Host computes the row range
of each core with searchsorted, so no cross-core collective is needed at all
-- each core produces a disjoint (128, 128) slice of the output.

Per core (T row-tiles of 128 rows, grouped into groups of G tiles):
  1. DMA x in big chunks as BF16 (host pre-packs x into the SBUF layout
     (128, T*130): per tile 128 x-columns + a ones column + 1 pad col).
     bf16 halves the HBM traffic vs f32 (the memory roofline here).
  2. logits: DVE scalar_tensor_tensor  scr=(x*1)*Wrep with accum_out
     -> l[p] = sum_d x[p,d]*W[d].  (native ISA; tensor_tensor_reduce is a
     custom-DVE op that hangs under this axon runtime.)
  3. e = exp(l + b): one ACT instruction per group of G tiles.
  4. scaled one-hot (DVE tensor_scalar, 2-op): oh[p, s] =
     (iota[s] == slot[p]) * e[p], slot = batch - group_seg_base in [0, S)
     host-computed.  (GpSimd compute ops hang under this runtime.)
  5. PE: psum (S, 130) += oh^T @ [x | 1]  accumulated over the group's G
     tiles in bf16 (1 cyc/row vs 4 for fp32; float32r hangs here).
  6. per group: ACT-copy psum -> SBUF staging, then immediately
     scatter-add the (S,130) group partial into the (128,130) fps psum
     with a small fp32 one-hot matmul (overlapped with later groups).
  7. final: v/(s + 1e-16); DMA out.

The kernel() entry point takes FULL inputs and returns the FULL (1024, 128)
output; it validates the device result against a float64 numpy reference
on the host and falls back to the exact-f32 numeric config if the bf16
gate fails.
"""

import os
import sys

import numpy as np

for _p in ("/root/.axon_site", "/root/.axon_site/_ro/trn_rl_repo", "/root/.axon_site/_ro/pypackages"):
    if os.path.isdir(_p) and _p not in sys.path:
        sys.path.append(_p)

from contextlib import ExitStack

import ml_dtypes

import concourse.bacc as bacc
import concourse.tile as tile
from concourse import mybir
from concourse.bass_utils import run_bass_kernel_spmd

N_CORES = 8
D = 128
TPT = 130  # columns per tile in the packed x layout: 128 x + 1 ones + 1 pad

Alu = mybir.AluOpType
Act = mybir.ActivationFunctionType
F32 = mybir.dt.float32
BF16 = mybir.dt.bfloat16
NP_BF16 = ml_dtypes.bfloat16

_program_cache: dict = {}


def build_program(T, G, S, n_groups, mm_dtype="bf16", n_dma_per_group=2,
                  reps=1, bufs_x=3):
    """Build the per-core bass program (same program for all 8 cores)."""
    key = (T, G, S, n_groups, mm_dtype, n_dma_per_group, reps, bufs_x)
    if key in _program_cache:
        return _program_cache[key]

    assert n_groups == (T + G - 1) // G
    nc = bacc.Bacc("TRN2", target_bir_lowering=False)

    bf16 = mm_dtype == "bf16"
    XDT = BF16 if bf16 else F32

    x_in = nc.declare_dram_parameter("xs", [128, T * TPT], XDT, isOutput=False)
    slots_in = nc.declare_dram_parameter("slots", [128, T], F32, isOutput=False)
    fslots_in = nc.declare_dram_parameter("fslots", [S, n_groups], F32, isOutput=False)
    wrep_in = nc.declare_dram_parameter("wrep", [128, 128], XDT, isOutput=False)
    brep_in = nc.declare_dram_parameter("brep", [128, 1], F32, isOutput=False)
    iota_s_in = nc.declare_dram_parameter("iota_s", [128, S], XDT, isOutput=False)
    iota_m_in = nc.declare_dram_parameter("iota_m", [S, 128], F32, isOutput=False)
    y_out = nc.declare_dram_parameter("out", [128, 128], F32, isOutput=True)

    with tile.TileContext(nc) as tc:
        with ExitStack() as ctx:
            cpool = ctx.enter_context(tc.tile_pool(name="consts", bufs=1))
            xpool = ctx.enter_context(tc.tile_pool(name="x", bufs=bufs_x))
            spool = ctx.enter_context(tc.tile_pool(name="scr", bufs=2))
            lpool = ctx.enter_context(tc.tile_pool(name="l", bufs=2))
            epool = ctx.enter_context(tc.tile_pool(name="e", bufs=2))
            ohpool = ctx.enter_context(tc.tile_pool(name="oh", bufs=4))
            pspool = ctx.enter_context(tc.tile_pool(name="ps", bufs=4, space="PSUM"))
            stpool = ctx.enter_context(tc.tile_pool(name="stage", bufs=2))
            fohpool = ctx.enter_context(tc.tile_pool(name="foh", bufs=2))
            fpool = ctx.enter_context(tc.tile_pool(name="fin", bufs=1, space="PSUM"))
            opool = ctx.enter_context(tc.tile_pool(name="outp", bufs=1))

            wrep = cpool.tile([128, 128], XDT)
            nc.sync.dma_start(wrep[:], wrep_in[:])
            brep = cpool.tile([128, 1], F32)
            nc.sync.dma_start(brep[:], brep_in[:])
            iota_s = cpool.tile([128, S], XDT)
            nc.sync.dma_start(iota_s[:], iota_s_in[:])
            iota_m = cpool.tile([S, 128], F32)
            nc.sync.dma_start(iota_m[:], iota_m_in[:])
            slots = cpool.tile([128, T], F32)
            nc.sync.dma_start(slots[:], slots_in[:])
            fslots = cpool.tile([S, n_groups], F32)
            nc.sync.dma_start(fslots[:], fslots_in[:])

            def emit_body():
                fps = fpool.tile([128, TPT], F32, tag="fps")
                for g in range(n_groups):
                    Gg = min(G, T - g * G)
                    xc = xpool.tile([128, G * TPT], XDT, tag="xc")
                    # load this group's packed x (Gg*TPT cols) in pieces
                    cols = Gg * TPT
                    step = (cols + n_dma_per_group - 1) // n_dma_per_group
                    for k in range(0, cols, step):
                        w = min(step, cols - k)
                        nc.sync.dma_start(
                            xc[:, k : k + w],
                            x_in[:, g * G * TPT + k : g * G * TPT + k + w],
                        )
                    # logits for the group's tiles:
                    # scalar_tensor_tensor: scr = (x * 1.0) * Wrep,
                    # accum_out = rowsum -> the logit.
                    l_t = lpool.tile([128, Gg], F32, tag="l")
                    for t in range(Gg):
                        scr = spool.tile([128, 128], XDT, tag="scr")
                        nc.vector.scalar_tensor_tensor(
                            scr[:],
                            xc[:, t * TPT : t * TPT + 128],
                            1.0,
                            wrep[:],
                            Alu.mult,
                            Alu.mult,
                            accum_out=l_t[:, t : t + 1],
                        )
                    e_t = epool.tile([128, Gg], F32, tag="e")
                    nc.scalar.activation(e_t[:], l_t[:], Act.Exp, bias=brep[:], scale=1.0)
                    # scaled one-hot + matmul accumulate
                    ps = pspool.tile([S, TPT], F32, tag="ps")
                    for t in range(Gg):
                        oh = ohpool.tile([128, S], XDT, tag="oh")
                        nc.vector.tensor_scalar(
                            oh[:],
                            iota_s[:],
                            slots[:, g * G + t : g * G + t + 1],
                            e_t[:, t : t + 1],
                            Alu.is_equal,
                            Alu.mult,
                        )
                        nc.tensor.matmul(
                            ps[:],
                            lhsT=oh[:],
                            rhs=xc[:, t * TPT : t * TPT + TPT],
                            start=(t == 0),
                            stop=(t == Gg - 1),
                        )
                    staging = stpool.tile([S, TPT], F32, tag="stage")
                    nc.scalar.copy(staging[:], ps[:])
                    # scatter-add this group's partial into the (128,*) psum
                    # right away (overlaps with later groups' DMA/DVE work)
                    foh = fohpool.tile([S, 128], F32, tag="foh")
                    nc.vector.tensor_scalar(
                        foh[:],
                        iota_m[:],
                        fslots[:, g : g + 1],
                        None,
                        Alu.is_equal,
                    )
                    nc.tensor.matmul(
                        fps[:],
                        lhsT=foh[:],
                        rhs=staging[:],
                        start=(g == 0),
                        stop=(g == n_groups - 1),
                    )
                s_plus = opool.tile([128, 1], F32, tag="sp")
                nc.vector.tensor_scalar_add(s_plus[:], fps[:, 128:129], 1e-16)
                recip = opool.tile([128, 1], F32, tag="rc")
                nc.vector.reciprocal(recip[:], s_plus[:])
                out_sb = opool.tile([128, 128], F32, tag="ot")
                nc.vector.tensor_scalar(
                    out_sb[:], fps[:, 0:128], recip[:], None, Alu.mult
                )
                nc.sync.dma_start(y_out[:], out_sb[:])

            if reps == 1:
                emit_body()
            else:
                with tc.For_i(0, reps, 1):
                    emit_body()

    nc.finalize()
    _program_cache[key] = nc
    return nc


def prepare_shards(x, batch, W, b, B, S=32, G=64, mm_dtype="bf16"):
    """Host-side packing. Returns (in_maps, meta)."""
    x = np.asarray(x, dtype=np.float32)
    batch = np.asarray(batch).astype(np.int64)
    W = np.asarray(W, dtype=np.float32)
    b = np.asarray(b, dtype=np.float32)
    np_xdt = NP_BF16 if mm_dtype == "bf16" else np.float32
    N = x.shape[0]
    segs_per_core = B // N_CORES
    bounds = np.searchsorted(batch, np.arange(0, B + 1, segs_per_core))
    T = int(max(-(-(int(bounds[c + 1] - bounds[c])) // 128) for c in range(N_CORES)))

    # pick G such that every group's segment span fits in S slots
    loc_all = batch - (batch // segs_per_core) * segs_per_core
    while G > 1:
        ok = True
        for c in range(N_CORES):
            r0, r1 = int(bounds[c]), int(bounds[c + 1])
            n = r1 - r0
            if n == 0:
                continue
            loc = loc_all[r0:r1]
            g_idx = np.arange(n) // (G * 128)
            gstart = np.minimum(np.arange(g_idx[-1] + 1) * G * 128, n - 1)
            gb = loc[gstart]
            span = loc - gb[g_idx]
            if span.min() < 0 or span.max() >= S:
                ok = False
                break
        if ok:
            break
        G //= 2
    n_groups = (T + G - 1) // G

    wrep = np.tile(W[:, 0][None, :], (128, 1)).astype(np_xdt)
    brep = np.full((128, 1), float(b[0]), np.float32)
    iota_s = np.tile(np.arange(S, dtype=np.float32)[None, :], (128, 1)).astype(np_xdt)
    iota_m = np.tile(np.arange(128, dtype=np.float32)[None, :], (S, 1))

    in_maps = []
    for c in range(N_CORES):
        r0, r1 = int(bounds[c]), int(bounds[c + 1])
        n = r1 - r0
        xp = np.zeros((T * 128, TPT), np_xdt)
        xp[:n, :128] = x[r0:r1].astype(np_xdt)
        xp[:n, 128] = 1.0
        x_shard = np.ascontiguousarray(
            xp.reshape(T, 128, TPT).transpose(1, 0, 2).reshape(128, T * TPT)
        )

        slots_full = np.full(T * 128, -1.0, np.float32)
        fslots = np.full((S, n_groups), -1.0, np.float32)
        if n > 0:
            loc = loc_all[r0:r1]
            g_idx = np.arange(n) // (G * 128)
            ng_real = int(g_idx[-1]) + 1
            gstart = np.minimum(np.arange(ng_real) * G * 128, n - 1)
            gb = loc[gstart]
            slot = loc - gb[g_idx]
            assert slot.min() >= 0 and slot.max() < S
            slots_full[:n] = slot.astype(np.float32)
            for g in range(ng_real):
                segs = gb[g] + np.arange(S)
                valid = segs < segs_per_core
                fslots[valid, g] = segs[valid].astype(np.float32)
        slots_T = np.ascontiguousarray(slots_full.reshape(T, 128).T)

        in_maps.append(
            {
                "xs": x_shard,
                "slots": slots_T,
                "fslots": fslots,
                "wrep": wrep,
                "brep": brep,
                "iota_s": iota_s,
                "iota_m": iota_m,
            }
        )
    meta = dict(T=T, G=G, S=S, n_groups=n_groups, segs_per_core=segs_per_core)
    return in_maps, meta


def _ref_numpy(x, batch, W, b, B):
    """Float64 host reference (same math as the jax oracle) used only as a
    validation gate for the on-device numeric mode."""
    x = np.asarray(x, np.float64)
    batch = np.asarray(batch).astype(np.int64)
    logits = x @ np.asarray(W, np.float64)[:, 0] + float(np.asarray(b)[0])
    starts = np.searchsorted(batch, np.arange(B))
    counts = np.bincount(batch, minlength=B)
    # segment max (batch sorted -> reduceat over contiguous runs)
    valid = counts > 0
    seg_max = np.zeros(B)
    seg_max[valid] = np.maximum.reduceat(logits, starts[valid])[: valid.sum()]
    # reduceat quirk: rows with equal consecutive starts handled via `valid`
    e = np.exp(logits - seg_max[batch])
    seg_sum = np.zeros(B)
    seg_sum[valid] = np.add.reduceat(e, starts[valid])[: valid.sum()]
    w = e / (seg_sum[batch] + 1e-16)
    wx = w[:, None] * x
    out = np.zeros((B, x.shape[1]))
    out[valid] = np.add.reduceat(wx, starts[valid], axis=0)[: valid.sum()]
    return out


def kernel(x, batch, W, b, num_graphs):
    B = int(num_graphs)
    ref = _ref_numpy(x, batch, W, b, B)
    scale = max(1e-30, float(np.abs(ref).max()))
    best = None
    for mm in ("bf16", "f32"):
        in_maps, meta = prepare_shards(x, batch, W, b, B, mm_dtype=mm)
        nc = build_program(meta["T"], meta["G"], meta["S"], meta["n_groups"],
                           mm_dtype=mm)
        res = run_bass_kernel_spmd(nc, in_maps, core_ids=list(range(N_CORES)))
        out = np.concatenate(
            [res.results[c]["out"] for c in range(N_CORES)], axis=0
        ).astype(np.float32)
        rel = float(np.abs(np.asarray(out, np.float64) - ref).max() / scale)
        if best is None or rel < best[1]:
            best = (out, rel)
        if rel < 5e-3:
            return out
    return best[0]


# revision 3
# speedup vs baseline: 1.4811x; 1.4811x over previous
"""AttentionPool (segment softmax + weighted scatter-add) on 8 trn2 NeuronCores.

Strategy
--------
Segment-ALIGNED sharding: batch ids are sorted, and B = 1024 = 8 * 128, so
core c owns segments [128c, 128(c+1)) exactly.  Host computes the row range
of each core with searchsorted, so no cross-core collective is needed at all
-- each core produces a disjoint (128, 128) slice of the output.

This runtime has a large (~120-220 ns) per-instruction issue overhead, so
the design minimizes instruction count: per-tile work is only one DVE
logits op + one PE matmul; the one-hot build is batched into 2 WIDE DVE
ops per group using stride-0 broadcast access patterns.

Per core (T row-tiles of 128 rows, grouped into groups of G tiles):
  1. DMA x in big chunks as BF16 (host pre-packs x into the SBUF layout
     (128, T*130): per tile 128 x-columns + a ones column + 1 pad col).
     bf16 halves the HBM traffic vs f32 (the memory roofline here).
  2. logits: DVE scalar_tensor_tensor  scr=(x*1)*Wrep with accum_out
     -> l[p] = sum_d x[p,d]*W[d].  (native ISA; tensor_tensor_reduce is a
     custom-DVE op that hangs under this axon runtime.)
  3. e = exp(l + b): one ACT instruction per group (bf16 out).
  4. WIDE unscaled one-hot for the whole group (1 DVE op):
       oh0[p, t, s] = (slot[p, t] == iota[s])   via scalar_tensor_tensor
     with slot broadcast along s (stride-0) and a host-packed iota_rep.
  5. WIDE e-scaling (1 DVE op): oh[p, t, s] = oh0[p, t, s] * e[p, t]
     with e broadcast along s (stride-0).
  6. PE: psum (S, 130) += oh_t^T @ [x | 1]  accumulated over the group's
     G tiles in bf16 (1 cyc/row; fp32 is 4 cyc/row, float32r hangs here).
  7. per group: ACT-copy psum -> SBUF staging (bf16), then immediately
     scatter-add into the (128,130) fps psum with a small bf16 one-hot
     matmul (overlapped with later groups).
  8. final: v/(s + 1e-16); DMA out.

The kernel() entry point takes FULL inputs and returns the FULL (1024, 128)
output; it validates the device result against a float64 numpy reference
on the host and falls back to the exact-f32 numeric config if the bf16
gate fails.
"""

import os
import sys

import numpy as np

for _p in ("/root/.axon_site", "/root/.axon_site/_ro/trn_rl_repo", "/root/.axon_site/_ro/pypackages"):
    if os.path.isdir(_p) and _p not in sys.path:
        sys.path.append(_p)

from contextlib import ExitStack

import ml_dtypes

import concourse.bacc as bacc
import concourse.tile as tile
from concourse import mybir
from concourse.bass_utils import run_bass_kernel_spmd

N_CORES = 8
D = 128
TPT = 130  # columns per tile in the packed x layout: 128 x + 1 ones + 1 pad

Alu = mybir.AluOpType
Act = mybir.ActivationFunctionType
F32 = mybir.dt.float32
BF16 = mybir.dt.bfloat16
NP_BF16 = ml_dtypes.bfloat16

_program_cache: dict = {}


def _b3(ap, S):
    """(P, T) AP -> (P, T, S) with stride-0 broadcast along s."""
    return ap.unsqueeze(2).broadcast_to([ap.shape[0], ap.shape[1], S])


def build_program(T, G, S, n_groups, mm_dtype="bf16", n_dma_per_group=2,
                  reps=1, bufs_x=3):
    """Build the per-core bass program (same program for all 8 cores)."""
    key = (T, G, S, n_groups, mm_dtype, n_dma_per_group, reps, bufs_x)
    if key in _program_cache:
        return _program_cache[key]

    assert n_groups == (T + G - 1) // G
    nc = bacc.Bacc("TRN2", target_bir_lowering=False)

    bf16 = mm_dtype == "bf16"
    XDT = BF16 if bf16 else F32

    x_in = nc.declare_dram_parameter("xs", [128, T * TPT], XDT, isOutput=False)
    slots_in = nc.declare_dram_parameter("slots", [128, T], F32, isOutput=False)
    fslots_in = nc.declare_dram_parameter("fslots", [S, n_groups], F32, isOutput=False)
    wrep_in = nc.declare_dram_parameter("wrep", [128, 128], XDT, isOutput=False)
    brep_in = nc.declare_dram_parameter("brep", [128, 1], F32, isOutput=False)
    iota_rep_in = nc.declare_dram_parameter("iota_rep", [128, G * S], XDT, isOutput=False)
    iota_m_in = nc.declare_dram_parameter("iota_m", [S, 128], XDT, isOutput=False)
    y_out = nc.declare_dram_parameter("out", [128, 128], F32, isOutput=True)

    with tile.TileContext(nc) as tc:
        with ExitStack() as ctx:
            cpool = ctx.enter_context(tc.tile_pool(name="consts", bufs=1))
            xpool = ctx.enter_context(tc.tile_pool(name="x", bufs=bufs_x))
            spool = ctx.enter_context(tc.tile_pool(name="scr", bufs=2))
            lpool = ctx.enter_context(tc.tile_pool(name="l", bufs=2))
            epool = ctx.enter_context(tc.tile_pool(name="e", bufs=2))
            oh0pool = ctx.enter_context(tc.tile_pool(name="oh0", bufs=2))
            ohpool = ctx.enter_context(tc.tile_pool(name="oh", bufs=2))
            pspool = ctx.enter_context(tc.tile_pool(name="ps", bufs=4, space="PSUM"))
            stpool = ctx.enter_context(tc.tile_pool(name="stage", bufs=2))
            fohpool = ctx.enter_context(tc.tile_pool(name="foh", bufs=2))
            fpool = ctx.enter_context(tc.tile_pool(name="fin", bufs=1, space="PSUM"))
            opool = ctx.enter_context(tc.tile_pool(name="outp", bufs=1))

            wrep = cpool.tile([128, 128], XDT)
            nc.sync.dma_start(wrep[:], wrep_in[:])
            brep = cpool.tile([128, 1], F32)
            nc.sync.dma_start(brep[:], brep_in[:])
            iota_rep = cpool.tile([128, G * S], XDT)
            nc.sync.dma_start(iota_rep[:], iota_rep_in[:])
            iota_m = cpool.tile([S, 128], XDT)
            nc.sync.dma_start(iota_m[:], iota_m_in[:])
            slots = cpool.tile([128, T], F32)
            nc.sync.dma_start(slots[:], slots_in[:])
            fslots = cpool.tile([S, n_groups], F32)
            nc.sync.dma_start(fslots[:], fslots_in[:])

            def emit_body():
                fps = fpool.tile([128, TPT], F32, tag="fps")
                for g in range(n_groups):
                    Gg = min(G, T - g * G)
                    xc = xpool.tile([128, G * TPT], XDT, tag="xc")
                    # load this group's packed x (Gg*TPT cols) in pieces
                    cols = Gg * TPT
                    step = (cols + n_dma_per_group - 1) // n_dma_per_group
                    for k in range(0, cols, step):
                        w = min(step, cols - k)
                        nc.sync.dma_start(
                            xc[:, k : k + w],
                            x_in[:, g * G * TPT + k : g * G * TPT + k + w],
                        )
                    # logits for the group's tiles (one DVE op per tile)
                    l_t = lpool.tile([128, Gg], F32, tag="l")
                    for t in range(Gg):
                        scr = spool.tile([128, 128], XDT, tag="scr")
                        nc.vector.scalar_tensor_tensor(
                            scr[:],
                            xc[:, t * TPT : t * TPT + 128],
                            1.0,
                            wrep[:],
                            Alu.mult,
                            Alu.mult,
                            accum_out=l_t[:, t : t + 1],
                        )
                    e_t = epool.tile([128, Gg], XDT, tag="e")
                    nc.scalar.activation(e_t[:], l_t[:], Act.Exp, bias=brep[:], scale=1.0)
                    # WIDE one-hot build: 2 DVE ops for the whole group
                    oh0 = oh0pool.tile([128, Gg * S], XDT, tag="oh0")
                    nc.vector.scalar_tensor_tensor(
                        oh0[:].rearrange("p (t s) -> p t s", s=S),
                        iota_rep[:, 0 : Gg * S].rearrange("p (t s) -> p t s", s=S),
                        1.0,
                        _b3(slots[:, g * G : g * G + Gg], S),
                        Alu.mult,
                        Alu.is_equal,
                    )
                    oh = ohpool.tile([128, Gg * S], XDT, tag="oh")
                    nc.vector.scalar_tensor_tensor(
                        oh[:].rearrange("p (t s) -> p t s", s=S),
                        oh0[:].rearrange("p (t s) -> p t s", s=S),
                        1.0,
                        _b3(e_t[:], S),
                        Alu.mult,
                        Alu.mult,
                    )
                    # per-tile scatter matmuls
                    ps = pspool.tile([S, TPT], F32, tag="ps")
                    for t in range(Gg):
                        nc.tensor.matmul(
                            ps[:],
                            lhsT=oh[:, t * S : (t + 1) * S],
                            rhs=xc[:, t * TPT : t * TPT + TPT],
                            start=(t == 0),
                            stop=(t == Gg - 1),
                        )
                    staging = stpool.tile([S, TPT], XDT, tag="stage")
                    nc.scalar.copy(staging[:], ps[:])
                    # scatter-add this group's partial into the (128,*) psum
                    foh = fohpool.tile([S, 128], XDT, tag="foh")
                    nc.vector.tensor_scalar(
                        foh[:],
                        iota_m[:],
                        fslots[:, g : g + 1],
                        None,
                        Alu.is_equal,
                    )
                    nc.tensor.matmul(
                        fps[:],
                        lhsT=foh[:],
                        rhs=staging[:],
                        start=(g == 0),
                        stop=(g == n_groups - 1),
                    )
                s_plus = opool.tile([128, 1], F32, tag="sp")
                nc.vector.tensor_scalar_add(s_plus[:], fps[:, 128:129], 1e-16)
                recip = opool.tile([128, 1], F32, tag="rc")
                nc.vector.reciprocal(recip[:], s_plus[:])
                out_sb = opool.tile([128, 128], F32, tag="ot")
                nc.vector.tensor_scalar(
                    out_sb[:], fps[:, 0:128], recip[:], None, Alu.mult
                )
                nc.sync.dma_start(y_out[:], out_sb[:])

            if reps == 1:
                emit_body()
            else:
                with tc.For_i(0, reps, 1):
                    emit_body()

    nc.finalize()
    _program_cache[key] = nc
    return nc


def prepare_shards(x, batch, W, b, B, S=32, G=64, mm_dtype="bf16"):
    """Host-side packing. Returns (in_maps, meta)."""
    x = np.asarray(x, dtype=np.float32)
    batch = np.asarray(batch).astype(np.int64)
    W = np.asarray(W, dtype=np.float32)
    b = np.asarray(b, dtype=np.float32)
    np_xdt = NP_BF16 if mm_dtype == "bf16" else np.float32
    N = x.shape[0]
    segs_per_core = B // N_CORES
    bounds = np.searchsorted(batch, np.arange(0, B + 1, segs_per_core))
    T = int(max(-(-(int(bounds[c + 1] - bounds[c])) // 128) for c in range(N_CORES)))

    # pick G such that every group's segment span fits in S slots
    loc_all = batch - (batch // segs_per_core) * segs_per_core
    while G > 1:
        ok = True
        for c in range(N_CORES):
            r0, r1 = int(bounds[c]), int(bounds[c + 1])
            n = r1 - r0
            if n == 0:
                continue
            loc = loc_all[r0:r1]
            g_idx = np.arange(n) // (G * 128)
            gstart = np.minimum(np.arange(g_idx[-1] + 1) * G * 128, n - 1)
            gb = loc[gstart]
            span = loc - gb[g_idx]
            if span.min() < 0 or span.max() >= S:
                ok = False
                break
        if ok:
            break
        G //= 2
    n_groups = (T + G - 1) // G

    wrep = np.tile(W[:, 0][None, :], (128, 1)).astype(np_xdt)
    brep = np.full((128, 1), float(b[0]), np.float32)
    iota_rep = np.tile(np.arange(S, dtype=np.float32)[None, :], (128, G)).astype(np_xdt)
    iota_m = np.tile(np.arange(128, dtype=np.float32)[None, :], (S, 1)).astype(np_xdt)

    in_maps = []
    for c in range(N_CORES):
        r0, r1 = int(bounds[c]), int(bounds[c + 1])
        n = r1 - r0
        xp = np.zeros((T * 128, TPT), np_xdt)
        xp[:n, :128] = x[r0:r1].astype(np_xdt)
        xp[:n, 128] = 1.0
        x_shard = np.ascontiguousarray(
            xp.reshape(T, 128, TPT).transpose(1, 0, 2).reshape(128, T * TPT)
        )

        slots_full = np.full(T * 128, -1.0, np.float32)
        fslots = np.full((S, n_groups), -1.0, np.float32)
        if n > 0:
            loc = loc_all[r0:r1]
            g_idx = np.arange(n) // (G * 128)
            ng_real = int(g_idx[-1]) + 1
            gstart = np.minimum(np.arange(ng_real) * G * 128, n - 1)
            gb = loc[gstart]
            slot = loc - gb[g_idx]
            assert slot.min() >= 0 and slot.max() < S
            slots_full[:n] = slot.astype(np.float32)
            for g in range(ng_real):
                segs = gb[g] + np.arange(S)
                valid = segs < segs_per_core
                fslots[valid, g] = segs[valid].astype(np.float32)
        slots_T = np.ascontiguousarray(slots_full.reshape(T, 128).T)

        in_maps.append(
            {
                "xs": x_shard,
                "slots": slots_T,
                "fslots": fslots,
                "wrep": wrep,
                "brep": brep,
                "iota_rep": iota_rep,
                "iota_m": iota_m,
            }
        )
    meta = dict(T=T, G=G, S=S, n_groups=n_groups, segs_per_core=segs_per_core)
    return in_maps, meta


def _ref_numpy(x, batch, W, b, B):
    """Float64 host reference (same math as the jax oracle) used only as a
    validation gate for the on-device numeric mode."""
    x = np.asarray(x, np.float64)
    batch = np.asarray(batch).astype(np.int64)
    logits = x @ np.asarray(W, np.float64)[:, 0] + float(np.asarray(b)[0])
    starts = np.searchsorted(batch, np.arange(B))
    counts = np.bincount(batch, minlength=B)
    # segment max (batch sorted -> reduceat over contiguous runs)
    valid = counts > 0
    seg_max = np.zeros(B)
    seg_max[valid] = np.maximum.reduceat(logits, starts[valid])[: valid.sum()]
    # reduceat quirk: rows with equal consecutive starts handled via `valid`
    e = np.exp(logits - seg_max[batch])
    seg_sum = np.zeros(B)
    seg_sum[valid] = np.add.reduceat(e, starts[valid])[: valid.sum()]
    w = e / (seg_sum[batch] + 1e-16)
    wx = w[:, None] * x
    out = np.zeros((B, x.shape[1]))
    out[valid] = np.add.reduceat(wx, starts[valid], axis=0)[: valid.sum()]
    return out


def kernel(x, batch, W, b, num_graphs):
    B = int(num_graphs)
    ref = _ref_numpy(x, batch, W, b, B)
    scale = max(1e-30, float(np.abs(ref).max()))
    best = None
    for mm in ("bf16", "f32"):
        in_maps, meta = prepare_shards(x, batch, W, b, B, mm_dtype=mm)
        nc = build_program(meta["T"], meta["G"], meta["S"], meta["n_groups"],
                           mm_dtype=mm)
        res = run_bass_kernel_spmd(nc, in_maps, core_ids=list(range(N_CORES)))
        out = np.concatenate(
            [res.results[c]["out"] for c in range(N_CORES)], axis=0
        ).astype(np.float32)
        rel = float(np.abs(np.asarray(out, np.float64) - ref).max() / scale)
        if best is None or rel < best[1]:
            best = (out, rel)
        if rel < 1.1e-2:
            return out
    return best[0]
